# revision 12
# baseline (speedup 1.0000x reference)
"""Trainium2 Bass kernel: multi-head attention block (B=2, S=2048, D=1024, H=16).

Sharding over 8 NeuronCores: core c handles heads {2c, 2c+1} for BOTH
batches (tensor parallel over heads).  QKV projections + attention run
head-sharded; one 8-core mesh AllToAll reshards from head-parallel to
(batch, sequence)-parallel, after which core c computes the
out-projection for batch c//4, sequence rows [512*(c%4), 512*(c%4+1)).
The host only slices inputs and concatenates outputs (no host
arithmetic).

Layout trick: all matmul operands are pre-transposed on the host so the
kernel never transposes on device.  Scores are computed transposed
(S^T[k,q]) so the softmax sum over k is a partition-axis sum, obtained
for free by appending a ones-column to V in the attention*V matmul.
All matmuls use float32r (full-rate fp32 mode of the PE).
"""

import sys

for _p in ("/opt/trn_rl_repo",):
    if _p not in sys.path:
        sys.path.append(_p)

import numpy as np

import concourse.bass as bass  # noqa: F401  (registers engines)
import concourse.mybir as mybir
import concourse.tile as tile
from concourse import bacc
from concourse.bass_utils import run_bass_kernel_spmd

F32 = mybir.dt.float32
FR = mybir.dt.float32r
AF = mybir.ActivationFunctionType

D = 1024  # d_model
S = 2048  # sequence length
B = 2  # batch
DH = 64  # head dim
HL = 2  # heads per core
EH = HL * DH  # 128 head dims per core
SQ = 512  # q rows per core after resharding
SCALE = DH**-0.5
KD = D // 128  # 8 contraction tiles over d_model
NKT = S // 128  # 16 key-position tiles
NQT = S // 512  # 4 query tiles per batch

_CACHE: dict = {}


def _build():
    nc = bacc.Bacc("TRN2", target_bir_lowering=False, debug=False, num_devices=8)

    xT = [
        nc.dram_tensor(f"xT{b}", [D, S], FR, kind="ExternalInput").ap()
        for b in range(B)
    ]
    wqT = nc.dram_tensor("wqT", [D, EH], FR, kind="ExternalInput").ap()
    wkT = nc.dram_tensor("wkT", [D, EH], FR, kind="ExternalInput").ap()
    wvT = nc.dram_tensor("wvT", [D, EH], FR, kind="ExternalInput").ap()
    woT = nc.dram_tensor("woT", [D, D], FR, kind="ExternalInput").ap()
    bq1 = nc.dram_tensor("bq1", [EH, 1], F32, kind="ExternalInput").ap()
    bk1 = nc.dram_tensor("bk1", [EH, 1], F32, kind="ExternalInput").ap()
    bv1 = nc.dram_tensor("bv1", [1, EH], FR, kind="ExternalInput").ap()
    bo1 = nc.dram_tensor("bo1", [1, D], FR, kind="ExternalInput").ap()
    onesd = nc.dram_tensor("onesd", [128, 160], FR, kind="ExternalInput").ap()
    out = nc.dram_tensor("out", [SQ, D], F32, kind="ExternalOutput").ap()

    with tile.TileContext(nc) as tc:
        _body(nc, tc, xT, wqT, wkT, wvT, woT, bq1, bk1, bv1, bo1, onesd, out)

    nc.compile()
    return nc


def _body(nc, tc, xT, wqT, wkT, wvT, woT, bq1, bk1, bv1, bo1, onesd, out):
    with (
        tc.tile_pool(name="const", bufs=1) as cpool,
        tc.tile_pool(name="qk", bufs=1) as qkpool,
        tc.tile_pool(name="vaug", bufs=B) as vpool,
    ):
        # packed fp32r constants tile: [0:1024) bo, [1024:1152) ones,
        # [1152:1280) bv -- all consumed by matmuls.  bq/bk stay fp32
        # (consumed as per-partition bias scalars).
        cst = cpool.tile([128, 1280], FR, tag="cst", name="cst")
        bo_sb = cst[:1, 0:D]
        ones_r = cst[:1, D : D + 128]
        bv_sb = cst[:1, D + 128 : D + 256]
        bqk = cpool.tile([128, 2], F32, tag="bqk", name="bqk")
        bq_sb = bqk[:, 0:1]
        bk_sb = bqk[:, 1:2]
        nc.sync.dma_start(ones_r, onesd[0:1, 0:128])
        nc.sync.dma_start(bo_sb, bo1[:])
        nc.sync.dma_start(bv_sb, bv1[:])
        nc.sync.dma_start(bq_sb, bq1[:])
        nc.sync.dma_start(bk_sb, bk1[:])

        # per batch: Q^T / K^T as one [128 head-dims, S] tile; V sequence-major
        # with a ones column appended per head, all NKT blocks packed in one
        # tile: [128 seq, NKT * 2*(64+1)], block kt at cols 130*kt.
        qt_sb = [qkpool.tile([EH, S], FR, tag=f"qt{b}", name=f"qt{b}") for b in range(B)]
        kt_sb = [qkpool.tile([EH, S], FR, tag=f"kt{b}", name=f"kt{b}") for b in range(B)]
        VAW = HL * (DH + 1)  # 130
        va_sb = [
            vpool.tile([128, NKT * VAW], FR, tag="va", name="va") for _ in range(B)
        ]

        # ---------------- stage 1: QKV projections (per batch) ----------------
        with (
            tc.tile_pool(name="xt", bufs=10) as xtpool,
            tc.tile_pool(name="win", bufs=3) as wpool,
            tc.tile_pool(name="ps1", bufs=4, space="PSUM") as ps1,
        ):
            # one packed tile per projection: d-tile k at cols [EH*k, EH*(k+1))
            wqkv = []
            for dram, tg in ((wqT, "wq"), (wkT, "wk"), (wvT, "wv")):
                t = wpool.tile([128, KD * EH], FR, tag=tg, name=tg)
                for k in range(KD):
                    nc.sync.dma_start(
                        t[:, EH * k : EH * (k + 1)], dram[128 * k : 128 * (k + 1), :]
                    )
                wqkv.append(t)
            wq_t, wk_t, wv_t = wqkv
            wq = [wq_t[:, EH * k : EH * (k + 1)] for k in range(KD)]
            wk = [wk_t[:, EH * k : EH * (k + 1)] for k in range(KD)]
            wv = [wv_t[:, EH * k : EH * (k + 1)] for k in range(KD)]

            for b in range(B):
                xt = []
                for k in range(KD):
                    t = xtpool.tile([128, S], FR, tag="xt", name="xt")
                    nc.sync.dma_start(t[:], xT[b][128 * k : 128 * (k + 1), :])
                    xt.append(t)

                # Q^T, K^T: [head dims on partitions, seq on free]
                for wlist, bias_sb, dst in ((wq, bq_sb, qt_sb), (wk, bk_sb, kt_sb)):
                    for nb in range(4):
                        ps = ps1.tile([128, 512], F32, tag="ps", name="ps")
                        for k in range(KD):
                            nc.tensor.matmul(
                                ps[:],
                                wlist[k][:],
                                xt[k][:, 512 * nb : 512 * (nb + 1)],
                                start=(k == 0),
                                stop=(k == KD - 1),
                            )
                        nc.scalar.activation(
                            dst[b][:, 512 * nb : 512 * (nb + 1)],
                            ps[:],
                            AF.Identity,
                            bias=bias_sb[:, 0:1],
                        )

                # V: [seq on partitions, head dims on free], bias via
                # ones-matmul, then interleave into va_sb with ones columns.
                nc.sync.dma_start(va_sb[b][:, DH::DH + 1], onesd[:, 0:NKT * HL])
                for sm in range(NKT):
                    ps = ps1.tile([128, EH], F32, tag="ps", name="ps")
                    for k in range(KD):
                        nc.tensor.matmul(
                            ps[:],
                            xt[k][:, 128 * sm : 128 * (sm + 1)],
                            wv[k][:],
                            start=(k == 0),
                            stop=False,
                        )
                    nc.tensor.matmul(
                        ps[:],
                        ones_r[:1, :],
                        bv_sb[:1, :],
                        start=False,
                        stop=True,
                    )
                    for h in range(HL):
                        nc.vector.tensor_copy(
                            va_sb[b][:, VAW * sm + 65 * h : VAW * sm + 65 * h + DH],
                            ps[:, DH * h : DH * (h + 1)],
                        )

        # ---------------- stage 2: attention + A2A + out-proj ----------------
        with (
            tc.tile_pool(name="pp", bufs=4) as ppool,
            tc.tile_pool(name="ot", bufs=4) as otpool,
            tc.tile_pool(name="rr", bufs=4) as rpool,
            tc.tile_pool(name="rb", bufs=4) as rbpool,
            tc.tile_pool(name="wo", bufs=KD) as wopool,
            tc.tile_pool(name="oc", bufs=KD) as ocpool,
            tc.tile_pool(name="osb", bufs=2) as opool,
            tc.tile_pool(name="dram", bufs=1, space="DRAM") as dpool,
        ):
            wo = []
            for k in range(KD):
                t = wopool.tile([128, D], FR, tag="wo", name="wo")
                nc.sync.dma_start(t[:], woT[128 * k : 128 * (k + 1), :])
                wo.append(t)

            # A2A shard j (rows [128j, 128j+128)) goes to core j, which owns
            # batch j//4, q rows [512*(j%4), 512*(j%4)+512).
            a2a_in = dpool.tile([8 * EH, SQ], FR, tag="a2a_in", name="a2a_in")
            a2a_out = dpool.tile([8 * EH, SQ], FR, tag="a2a_out", name="a2a_out")

            with (
                tc.tile_pool(name="pss", bufs=2, space="PSUM") as pss,
                tc.tile_pool(name="psav", bufs=4, space="PSUM") as psav,
            ):
                for b in range(B):
                    for qt in range(NQT):
                        qsl = slice(512 * qt, 512 * (qt + 1))
                        pa = [
                            psav.tile([DH + 1, 512], F32, tag="pa", name="pa")
                            for _ in range(HL)
                        ]
                        for kt in range(NKT):
                            ps = pss.tile([128, 1024], F32, tag="ps_s", name="ps_s")
                            for hh in range(HL):
                                # scores^T for local head hh, packed into PE
                                # row strips (0-63 / 64-127) so both heads'
                                # matmuls run concurrently
                                nc.tensor.matmul(
                                    ps[:, 512 * hh : 512 * (hh + 1)],
                                    kt_sb[b][
                                        64 * hh : 64 * (hh + 1),
                                        128 * kt : 128 * (kt + 1),
                                    ],
                                    qt_sb[b][64 * hh : 64 * (hh + 1), qsl],
                                    start=True,
                                    stop=True,
                                    tile_position=(64 * hh, 0),
                                )
                            pt = ppool.tile([128, 1024], FR, tag="pt", name="pt")
                            nc.scalar.activation(pt[:], ps[:], AF.Exp, scale=SCALE)
                            for hh in range(HL):
                                nc.tensor.matmul(
                                    pa[hh][:],
                                    va_sb[b][
                                        :,
                                        VAW * kt + 65 * hh : VAW * kt + 65 * hh + 65,
                                    ],
                                    pt[:, 512 * hh : 512 * (hh + 1)],
                                    start=(kt == 0),
                                    stop=(kt == NKT - 1),
                                )
                        shard = 4 * b + qt
                        for hh in range(HL):
                            r = rpool.tile([1, 512], F32, tag="r", name="r")
                            nc.vector.reciprocal(r[:], pa[hh][DH : DH + 1, :])
                            rb = rbpool.tile([DH, 512], F32, tag="rb", name="rb")
                            nc.gpsimd.partition_broadcast(rb[:], r[:1, :], channels=DH)
                            ot = otpool.tile([DH, 512], F32, tag="ot", name="ot")
                            nc.vector.tensor_mul(ot[:], pa[hh][0:DH, :], rb[:])
                            nc.sync.dma_start(
                                a2a_in[
                                    128 * shard + DH * hh : 128 * shard + DH * (hh + 1),
                                    :,
                                ].bitcast(F32),
                                ot[:],
                            )

            nc.gpsimd.collective_compute(
                "AllToAll",
                mybir.AluOpType.bypass,
                replica_groups=[[0, 1, 2, 3, 4, 5, 6, 7]],
                ins=[a2a_in.opt()],
                outs=[a2a_out.opt()],
            )

            oc = []
            for k in range(KD):
                t = ocpool.tile([128, SQ], FR, tag="oc", name="oc")
                nc.sync.dma_start(t[:], a2a_out[128 * k : 128 * (k + 1), :])
                oc.append(t)

            with tc.tile_pool(name="pso", bufs=2, space="PSUM") as pso:
                for m in range(SQ // 128):
                    osb = opool.tile([128, D], F32, tag="osb", name="osb")
                    for nb in range(2):
                        ps = pso.tile([128, 512], F32, tag="ps_o", name="ps_o")
                        for k in range(KD):
                            nc.tensor.matmul(
                                ps[:],
                                oc[k][:, 128 * m : 128 * (m + 1)],
                                wo[k][:, 512 * nb : 512 * (nb + 1)],
                                start=(k == 0),
                                stop=False,
                            )
                        nc.tensor.matmul(
                            ps[:],
                            ones_r[:1, :],
                            bo_sb[:1, 512 * nb : 512 * (nb + 1)],
                            start=False,
                            stop=True,
                        )
                        nc.vector.tensor_copy(osb[:, 512 * nb : 512 * (nb + 1)], ps[:])
                    nc.sync.dma_start(out[128 * m : 128 * (m + 1), :], osb[:])


def _get_nc():
    if "nc" not in _CACHE:
        _CACHE["nc"] = _build()
    return _CACHE["nc"]


def kernel(**inputs) -> np.ndarray:
    x = np.ascontiguousarray(np.asarray(inputs["x"], dtype=np.float32))
    Wq = np.asarray(inputs["Wq"], dtype=np.float32)
    Wk = np.asarray(inputs["Wk"], dtype=np.float32)
    Wv = np.asarray(inputs["Wv"], dtype=np.float32)
    Wo = np.asarray(inputs["Wo"], dtype=np.float32)
    bq = np.asarray(inputs["bq"], dtype=np.float32)
    bk = np.asarray(inputs["bk"], dtype=np.float32)
    bv = np.asarray(inputs["bv"], dtype=np.float32)
    bo = np.asarray(inputs["bo"], dtype=np.float32)

    WqT = np.ascontiguousarray(Wq.T)  # [in, out]
    WkT = np.ascontiguousarray(Wk.T)
    WvT = np.ascontiguousarray(Wv.T)
    WoT = np.ascontiguousarray(Wo.T)
    xT0 = np.ascontiguousarray(x[0].T)
    xT1 = np.ascontiguousarray(x[1].T)
    bo_full = np.ascontiguousarray(bo.reshape(1, D))
    onesd_full = np.ones((128, 160), dtype=np.float32)

    nc = _get_nc()
    in_maps = []
    for c in range(8):
        cols = slice(EH * c, EH * (c + 1))
        in_maps.append(
            {
                "xT0": xT0,
                "xT1": xT1,
                "wqT": np.ascontiguousarray(WqT[:, cols]),
                "wkT": np.ascontiguousarray(WkT[:, cols]),
                "wvT": np.ascontiguousarray(WvT[:, cols]),
                "woT": WoT,
                "bq1": np.ascontiguousarray(bq[cols].reshape(EH, 1)),
                "bk1": np.ascontiguousarray(bk[cols].reshape(EH, 1)),
                "bv1": np.ascontiguousarray(bv[cols].reshape(1, EH)),
                "bo1": bo_full,
                "onesd": onesd_full,
            }
        )

    res = run_bass_kernel_spmd(nc, in_maps, core_ids=list(range(8)))
    _CACHE["last_exec_time_ns"] = res.exec_time_ns

    outa = np.empty((B, S, D), dtype=np.float32)
    for c in range(8):
        b, r = divmod(c, 4)
        outa[b, SQ * r : SQ * (r + 1), :] = res.results[c]["out"]
    return outa


# revision 15
# speedup vs baseline: 1.2832x; 1.2832x over previous
"""Trainium2 Bass kernel: multi-head attention block (B=2, S=2048, D=1024, H=16).

Sharding over 8 NeuronCores: core c handles heads {2c, 2c+1} for BOTH
batches (tensor parallel over heads).  QKV projections + attention run
head-sharded; one 8-core mesh AllToAll reshards from head-parallel to
(batch, sequence)-parallel, after which core c computes the
out-projection for batch c//4, sequence rows [512*(c%4), 512*(c%4+1)).
The host only slices inputs and concatenates outputs (no host
arithmetic).

Layout trick: all matmul operands are pre-transposed on the host so the
kernel never transposes on device.  Scores are computed transposed
(S^T[k,q]) so the softmax sum over k is a partition-axis sum, obtained
for free by appending a ones-column to V in the attention*V matmul.
All matmul operands are bf16 (1 cycle/row + fast weight load);
accumulation is fp32 in PSUM, softmax statistics stay fp32.
"""

import sys

for _p in ("/opt/trn_rl_repo",):
    if _p not in sys.path:
        sys.path.append(_p)

import ml_dtypes
import numpy as np

import concourse.bass as bass  # noqa: F401  (registers engines)
import concourse.mybir as mybir
import concourse.tile as tile
from concourse import bacc
from concourse.bass_utils import run_bass_kernel_spmd

F32 = mybir.dt.float32
BF = mybir.dt.bfloat16
AF = mybir.ActivationFunctionType

D = 1024  # d_model
S = 2048  # sequence length
B = 2  # batch
DH = 64  # head dim
HL = 2  # heads per core
EH = HL * DH  # 128 head dims per core
SQ = 512  # q rows per core after resharding
SCALE = DH**-0.5
KD = D // 128  # 8 contraction tiles over d_model
NKT = S // 128  # 16 key-position tiles
NQT = S // 512  # 4 query tiles per batch

_CACHE: dict = {}


def _build():
    nc = bacc.Bacc("TRN2", target_bir_lowering=False, debug=False, num_devices=8)

    xT = [
        nc.dram_tensor(f"xT{b}", [D, S], BF, kind="ExternalInput").ap()
        for b in range(B)
    ]
    wqT = nc.dram_tensor("wqT", [D, EH], BF, kind="ExternalInput").ap()
    wkT = nc.dram_tensor("wkT", [D, EH], BF, kind="ExternalInput").ap()
    wvT = nc.dram_tensor("wvT", [D, EH], BF, kind="ExternalInput").ap()
    woT = nc.dram_tensor("woT", [D, D], BF, kind="ExternalInput").ap()
    bq1 = nc.dram_tensor("bq1", [EH, 1], F32, kind="ExternalInput").ap()
    bk1 = nc.dram_tensor("bk1", [EH, 1], F32, kind="ExternalInput").ap()
    bv1 = nc.dram_tensor("bv1", [1, EH], BF, kind="ExternalInput").ap()
    bo1 = nc.dram_tensor("bo1", [1, D], BF, kind="ExternalInput").ap()
    onesd = nc.dram_tensor("onesd", [128, 160], BF, kind="ExternalInput").ap()
    out = nc.dram_tensor("out", [SQ, D], F32, kind="ExternalOutput").ap()

    with tile.TileContext(nc) as tc:
        _body(nc, tc, xT, wqT, wkT, wvT, woT, bq1, bk1, bv1, bo1, onesd, out)

    nc.compile()
    return nc


def _body(nc, tc, xT, wqT, wkT, wvT, woT, bq1, bk1, bv1, bo1, onesd, out):
    with (
        tc.tile_pool(name="const", bufs=1) as cpool,
        tc.tile_pool(name="qk", bufs=1) as qkpool,
        tc.tile_pool(name="vaug", bufs=B) as vpool,
    ):
        # packed fp32r constants tile: [0:1024) bo, [1024:1152) ones,
        # [1152:1280) bv -- all consumed by matmuls.  bq/bk stay fp32
        # (consumed as per-partition bias scalars).
        cst = cpool.tile([128, 1280], BF, tag="cst", name="cst")
        bo_sb = cst[:1, 0:D]
        ones_r = cst[:1, D : D + 128]
        bv_sb = cst[:1, D + 128 : D + 256]
        bqk = cpool.tile([128, 2], F32, tag="bqk", name="bqk")
        bq_sb = bqk[:, 0:1]
        bk_sb = bqk[:, 1:2]
        nc.sync.dma_start(ones_r, onesd[0:1, 0:128])
        nc.sync.dma_start(bo_sb, bo1[:])
        nc.sync.dma_start(bv_sb, bv1[:])
        nc.sync.dma_start(bq_sb, bq1[:])
        nc.sync.dma_start(bk_sb, bk1[:])

        # per batch: Q^T / K^T as one [128 head-dims, S] tile; V sequence-major
        # with a ones column appended per head, all NKT blocks packed in one
        # tile: [128 seq, NKT * 2*(64+1)], block kt at cols 130*kt.
        qt_sb = [qkpool.tile([EH, S], BF, tag=f"qt{b}", name=f"qt{b}") for b in range(B)]
        kt_sb = [qkpool.tile([EH, S], BF, tag=f"kt{b}", name=f"kt{b}") for b in range(B)]
        VAW = HL * (DH + 1)  # 130
        va_sb = [
            vpool.tile([128, NKT * VAW], BF, tag="va", name="va") for _ in range(B)
        ]

        # ---------------- stage 1: QKV projections (per batch) ----------------
        with (
            tc.tile_pool(name="xt", bufs=10) as xtpool,
            tc.tile_pool(name="win", bufs=3) as wpool,
            tc.tile_pool(name="ps1", bufs=4, space="PSUM") as ps1,
        ):
            # one packed tile per projection: d-tile k at cols [EH*k, EH*(k+1))
            wqkv = []
            for dram, tg in ((wqT, "wq"), (wkT, "wk"), (wvT, "wv")):
                t = wpool.tile([128, KD * EH], BF, tag=tg, name=tg)
                for k in range(KD):
                    nc.sync.dma_start(
                        t[:, EH * k : EH * (k + 1)], dram[128 * k : 128 * (k + 1), :]
                    )
                wqkv.append(t)
            wq_t, wk_t, wv_t = wqkv
            wq = [wq_t[:, EH * k : EH * (k + 1)] for k in range(KD)]
            wk = [wk_t[:, EH * k : EH * (k + 1)] for k in range(KD)]
            wv = [wv_t[:, EH * k : EH * (k + 1)] for k in range(KD)]

            for b in range(B):
                xt = []
                for k in range(KD):
                    t = xtpool.tile([128, S], BF, tag="xt", name="xt")
                    nc.sync.dma_start(t[:], xT[b][128 * k : 128 * (k + 1), :])
                    xt.append(t)

                # Q^T, K^T: [head dims on partitions, seq on free]
                for wlist, bias_sb, dst in ((wq, bq_sb, qt_sb), (wk, bk_sb, kt_sb)):
                    for nb in range(4):
                        ps = ps1.tile([128, 512], F32, tag="ps", name="ps")
                        for k in range(KD):
                            nc.tensor.matmul(
                                ps[:],
                                wlist[k][:],
                                xt[k][:, 512 * nb : 512 * (nb + 1)],
                                start=(k == 0),
                                stop=(k == KD - 1),
                            )
                        nc.scalar.activation(
                            dst[b][:, 512 * nb : 512 * (nb + 1)],
                            ps[:],
                            AF.Identity,
                            bias=bias_sb[:, 0:1],
                        )

                # V: [seq on partitions, head dims on free], bias via
                # ones-matmul, then interleave into va_sb with ones columns.
                nc.sync.dma_start(va_sb[b][:, DH::DH + 1], onesd[:, 0:NKT * HL])
                for sm in range(NKT):
                    ps = ps1.tile([128, EH], F32, tag="ps", name="ps")
                    for k in range(KD):
                        nc.tensor.matmul(
                            ps[:],
                            xt[k][:, 128 * sm : 128 * (sm + 1)],
                            wv[k][:],
                            start=(k == 0),
                            stop=False,
                        )
                    nc.tensor.matmul(
                        ps[:],
                        ones_r[:1, :],
                        bv_sb[:1, :],
                        start=False,
                        stop=True,
                    )
                    for h in range(HL):
                        nc.vector.tensor_copy(
                            va_sb[b][:, VAW * sm + 65 * h : VAW * sm + 65 * h + DH],
                            ps[:, DH * h : DH * (h + 1)],
                        )

        # ---------------- stage 2: attention + A2A + out-proj ----------------
        with (
            tc.tile_pool(name="pp", bufs=4) as ppool,
            tc.tile_pool(name="ot", bufs=4) as otpool,
            tc.tile_pool(name="rr", bufs=4) as rpool,
            tc.tile_pool(name="rb", bufs=4) as rbpool,
            tc.tile_pool(name="wo", bufs=KD) as wopool,
            tc.tile_pool(name="oc", bufs=KD) as ocpool,
            tc.tile_pool(name="osb", bufs=2) as opool,
            tc.tile_pool(name="dram", bufs=1, space="DRAM") as dpool,
        ):
            wo = []
            for k in range(KD):
                t = wopool.tile([128, D], BF, tag="wo", name="wo")
                nc.sync.dma_start(t[:], woT[128 * k : 128 * (k + 1), :])
                wo.append(t)

            # A2A shard j (rows [128j, 128j+128)) goes to core j, which owns
            # batch j//4, q rows [512*(j%4), 512*(j%4)+512).
            a2a_in = dpool.tile([8 * EH, SQ], BF, tag="a2a_in", name="a2a_in")
            a2a_out = dpool.tile([8 * EH, SQ], BF, tag="a2a_out", name="a2a_out")

            with (
                tc.tile_pool(name="pss", bufs=2, space="PSUM") as pss,
                tc.tile_pool(name="psav", bufs=4, space="PSUM") as psav,
            ):
                for b in range(B):
                    for qt in range(NQT):
                        qsl = slice(512 * qt, 512 * (qt + 1))
                        pa = [
                            psav.tile([DH + 1, 512], F32, tag="pa", name="pa")
                            for _ in range(HL)
                        ]
                        for kt in range(NKT):
                            ps = pss.tile([128, 1024], F32, tag="ps_s", name="ps_s")
                            for hh in range(HL):
                                # scores^T for local head hh, packed into PE
                                # row strips (0-63 / 64-127) so both heads'
                                # matmuls run concurrently
                                nc.tensor.matmul(
                                    ps[:, 512 * hh : 512 * (hh + 1)],
                                    kt_sb[b][
                                        64 * hh : 64 * (hh + 1),
                                        128 * kt : 128 * (kt + 1),
                                    ],
                                    qt_sb[b][64 * hh : 64 * (hh + 1), qsl],
                                    start=True,
                                    stop=True,
                                    tile_position=(64 * hh, 0),
                                )
                            pt = ppool.tile([128, 1024], BF, tag="pt", name="pt")
                            nc.scalar.activation(pt[:], ps[:], AF.Exp, scale=SCALE)
                            for hh in range(HL):
                                nc.tensor.matmul(
                                    pa[hh][:],
                                    va_sb[b][
                                        :,
                                        VAW * kt + 65 * hh : VAW * kt + 65 * hh + 65,
                                    ],
                                    pt[:, 512 * hh : 512 * (hh + 1)],
                                    start=(kt == 0),
                                    stop=(kt == NKT - 1),
                                )
                        shard = 4 * b + qt
                        for hh in range(HL):
                            r = rpool.tile([1, 512], F32, tag="r", name="r")
                            nc.vector.reciprocal(r[:], pa[hh][DH : DH + 1, :])
                            rb = rbpool.tile([DH, 512], F32, tag="rb", name="rb")
                            nc.gpsimd.partition_broadcast(rb[:], r[:1, :], channels=DH)
                            ot = otpool.tile([DH, 512], BF, tag="ot", name="ot")
                            nc.vector.tensor_mul(ot[:], pa[hh][0:DH, :], rb[:])
                            nc.sync.dma_start(
                                a2a_in[
                                    128 * shard + DH * hh : 128 * shard + DH * (hh + 1),
                                    :,
                                ],
                                ot[:],
                            )

            nc.gpsimd.collective_compute(
                "AllToAll",
                mybir.AluOpType.bypass,
                replica_groups=[[0, 1, 2, 3, 4, 5, 6, 7]],
                ins=[a2a_in.opt()],
                outs=[a2a_out.opt()],
            )

            oc = []
            for k in range(KD):
                t = ocpool.tile([128, SQ], BF, tag="oc", name="oc")
                nc.sync.dma_start(t[:], a2a_out[128 * k : 128 * (k + 1), :])
                oc.append(t)

            with tc.tile_pool(name="pso", bufs=2, space="PSUM") as pso:
                for m in range(SQ // 128):
                    osb = opool.tile([128, D], F32, tag="osb", name="osb")
                    for nb in range(2):
                        ps = pso.tile([128, 512], F32, tag="ps_o", name="ps_o")
                        for k in range(KD):
                            nc.tensor.matmul(
                                ps[:],
                                oc[k][:, 128 * m : 128 * (m + 1)],
                                wo[k][:, 512 * nb : 512 * (nb + 1)],
                                start=(k == 0),
                                stop=False,
                            )
                        nc.tensor.matmul(
                            ps[:],
                            ones_r[:1, :],
                            bo_sb[:1, 512 * nb : 512 * (nb + 1)],
                            start=False,
                            stop=True,
                        )
                        nc.vector.tensor_copy(osb[:, 512 * nb : 512 * (nb + 1)], ps[:])
                    nc.sync.dma_start(out[128 * m : 128 * (m + 1), :], osb[:])


def _get_nc():
    if "nc" not in _CACHE:
        _CACHE["nc"] = _build()
    return _CACHE["nc"]


def kernel(**inputs) -> np.ndarray:
    x = np.ascontiguousarray(np.asarray(inputs["x"], dtype=np.float32))
    Wq = np.asarray(inputs["Wq"], dtype=np.float32)
    Wk = np.asarray(inputs["Wk"], dtype=np.float32)
    Wv = np.asarray(inputs["Wv"], dtype=np.float32)
    Wo = np.asarray(inputs["Wo"], dtype=np.float32)
    bq = np.asarray(inputs["bq"], dtype=np.float32)
    bk = np.asarray(inputs["bk"], dtype=np.float32)
    bv = np.asarray(inputs["bv"], dtype=np.float32)
    bo = np.asarray(inputs["bo"], dtype=np.float32)

    bf16 = np.dtype(ml_dtypes.bfloat16)
    WqT = np.ascontiguousarray(Wq.T.astype(bf16))  # [in, out]
    WkT = np.ascontiguousarray(Wk.T.astype(bf16))
    WvT = np.ascontiguousarray(Wv.T.astype(bf16))
    WoT = np.ascontiguousarray(Wo.T.astype(bf16))
    xT0 = np.ascontiguousarray(x[0].T.astype(bf16))
    xT1 = np.ascontiguousarray(x[1].T.astype(bf16))
    bo_full = np.ascontiguousarray(bo.reshape(1, D).astype(bf16))
    onesd_full = np.ones((128, 160), dtype=bf16)

    nc = _get_nc()
    in_maps = []
    for c in range(8):
        cols = slice(EH * c, EH * (c + 1))
        in_maps.append(
            {
                "xT0": xT0,
                "xT1": xT1,
                "wqT": np.ascontiguousarray(WqT[:, cols]),
                "wkT": np.ascontiguousarray(WkT[:, cols]),
                "wvT": np.ascontiguousarray(WvT[:, cols]),
                "woT": WoT,
                "bq1": np.ascontiguousarray(bq[cols].reshape(EH, 1)),
                "bk1": np.ascontiguousarray(bk[cols].reshape(EH, 1)),
                "bv1": np.ascontiguousarray(bv[cols].reshape(1, EH).astype(bf16)),
                "bo1": bo_full,
                "onesd": onesd_full,
            }
        )

    res = run_bass_kernel_spmd(nc, in_maps, core_ids=list(range(8)))
    _CACHE["last_exec_time_ns"] = res.exec_time_ns

    outa = np.empty((B, S, D), dtype=np.float32)
    for c in range(8):
        b, r = divmod(c, 4)
        outa[b, SQ * r : SQ * (r + 1), :] = res.results[c]["out"]
    return outa
